# revision 42
# baseline (speedup 1.0000x reference)
"""Trainium2 Bass kernel for nn_Predictor_67585605370461 (segment_reduce).

Per patch (N=4194304, 9 elements each):
  m_dis = edg > 0.5 ; m_acc = (edg != 0) & ~m_dis
  md = mean(img | m_dis), ma = mean(img | m_acc)
  out = valid ? (md > ma ? 0 : 1) : 2
Global: avgB/avgW = masked means of center pixels over out==0 / out==1
  corr = out==2 ? (|v-avgB| < |v-avgW| ? 0 : 1) : out  -> reshape [2048,2048]

Key reductions used by the kernel:
  * md > ma  <=>  9*sd > c*sall   (sd = masked sum, c = count, sall = 9-sum)
  * exact zeros in the uniform edge data are measure-zero -> m_acc = ~m_dis
  * |v-avgB| >= |v-avgW|  <=>  (v - (avgB+avgW)/2) * (avgW-avgB) >= 0

Sharding: data-parallel over the patch axis, 524288 patches per core; the
four global scalars (sumB, sumW, cntB, cntW) go through one tiny AllReduce.
"""

import contextlib

import numpy as np

import concourse.bass as bass
import concourse.bacc as bacc
import concourse.mybir as mybir
import concourse.tile as tile
from concourse import bass_utils

N_CORES = 8
N_PATCH = 4194304
NP_CORE = N_PATCH // N_CORES  # 524288
P = 128
NINE = 9
W = 512                      # patches per partition-row per tile
F = W * NINE                 # 2304 free elements per tile
NTILES = NP_CORE // (P * W)  # 16
NPW = NTILES * W             # 4096 patches per partition per core
H_OUT = 2048
PH2_CHUNKS = 4
PH2_W = NPW // PH2_CHUNKS    # 1024

f32 = mybir.dt.float32
Alu = mybir.AluOpType
Act = mybir.ActivationFunctionType

_CACHE = {}
DEBUG_PROBES = False


def _build(stub_cc=False):
    num_devices = 1 if stub_cc else N_CORES
    nc = bacc.Bacc("TRN2", target_bir_lowering=False, debug=False,
                   num_devices=num_devices)
    img = nc.dram_tensor("img", [NP_CORE, NINE], f32, kind="ExternalInput")
    edg = nc.dram_tensor("edg", [NP_CORE, NINE], f32, kind="ExternalInput")
    out = nc.dram_tensor("out", [NP_CORE], f32, kind="ExternalOutput")
    if DEBUG_PROBES:
        d_outv = nc.dram_tensor("d_outv", [P, NPW], f32, kind="ExternalOutput")
        d_acc = nc.dram_tensor("d_acc", [P, 4], f32, kind="ExternalOutput")
        d_g = nc.dram_tensor("d_g", [1, 4], f32, kind="ExternalOutput")
        d_scb = nc.dram_tensor("d_scb", [P, 2], f32, kind="ExternalOutput")

    # partition-contiguous patch layout: partition p owns patches
    # [p*NPW, (p+1)*NPW) of this core's shard; tile i covers window
    # [i*W, (i+1)*W) of every partition.  Output is then one [P, NPW] DMA.
    img_t = img.ap().rearrange("(p i w) n -> i p (w n)", p=P, i=NTILES, w=W)
    edg_t = edg.ap().rearrange("(p i w) n -> i p (w n)", p=P, i=NTILES, w=W)
    out_f = out.ap().rearrange("(p j) -> p j", p=P)

    with tile.TileContext(nc) as tc:
        with (
            tc.tile_pool(name="vin", bufs=3) as vpool,
            tc.tile_pool(name="ein", bufs=3) as epool,
            tc.tile_pool(name="l1", bufs=2) as l1pool,
            tc.tile_pool(name="small", bufs=2) as spool,
            tc.tile_pool(name="red", bufs=3) as redpool,
            tc.tile_pool(name="persist", bufs=1) as pers,
            tc.tile_pool(name="psum", bufs=1, space="PSUM") as psum,
            tc.tile_pool(name="dram", bufs=1, space="DRAM") as dram,
        ):
            outv = pers.tile([P, NPW], f32)
            vals = pers.tile([P, NPW], f32)
            accU = pers.tile([P, NTILES], f32)   # per-tile unknown counts
            accO = pers.tile([P, NTILES], f32)   # per-tile sum of outv
            nbias = pers.tile([P, 1], f32)
            nc.vector.memset(nbias[:], -0.5)
            ones = pers.tile([1, P], f32)
            nc.vector.memset(ones[:], 1.0)

            for i in range(NTILES):
                Vt = vpool.tile([P, F], f32, tag="V")
                nc.sync.dma_start(Vt[:], img_t[i])
                Et = epool.tile([P, F], f32, tag="E")
                nc.sync.dma_start(Et[:], edg_t[i])

                # ACT: mask = sign(relu(E - 0.5)), computed in place on Et
                nc.scalar.activation(Et[:], Et[:], Act.Relu, bias=nbias[:])
                nc.scalar.sign(Et[:], Et[:])

                v3 = Vt[:].rearrange("p (w n) -> p w n", n=NINE)
                m3 = Et[:].rearrange("p (w n) -> p w n", n=NINE)

                # reads of the original V first: 9-sum + center pixel
                sall = redpool.tile([P, W], f32, tag="sall")
                nc.vector.tensor_reduce(
                    sall[:], v3, axis=mybir.AxisListType.X, op=Alu.add)
                nc.scalar.copy(vals[:, i * W:(i + 1) * W], v3[:, :, 4])

                # count tree: c = sum of 9 mask bits (L1 on gpsimd)
                l1 = l1pool.tile([P, W, 4], f32, tag="l1")
                nc.gpsimd.tensor_tensor(l1[:], m3[:, :, 0:4], m3[:, :, 4:8],
                                        op=Alu.add)

                # GpSimd: masked values, in place on Vt
                nc.gpsimd.tensor_tensor(Vt[:], Et[:], Vt[:], op=Alu.mult)
                sd = redpool.tile([P, W], f32, tag="sd")
                nc.vector.tensor_reduce(
                    sd[:], v3, axis=mybir.AxisListType.X, op=Alu.add)

                l2 = spool.tile([P, W, 2], f32, tag="l2")
                nc.vector.tensor_tensor(l2[:], l1[:, :, 0:2], l1[:, :, 2:4],
                                        op=Alu.add)
                nc.vector.tensor_tensor(l2[:, :, 0:1], l2[:, :, 0:1],
                                        l2[:, :, 1:2], op=Alu.add)
                ct = redpool.tile([P, W], f32, tag="ct")
                nc.vector.tensor_tensor(ct[:], l2[:, :, 0], m3[:, :, 8],
                                        op=Alu.add)

                # classification: out01 = (9*sd <= c*sall)
                # invalid iff (c-9)*c == 0
                t1 = redpool.tile([P, W], f32, tag="t1")
                nc.vector.tensor_tensor(t1[:], ct[:], sall[:], op=Alu.mult)
                out01 = redpool.tile([P, W], f32, tag="out01")
                nc.vector.scalar_tensor_tensor(out01[:], sd[:], 9.0, t1[:],
                                               op0=Alu.mult, op1=Alu.is_le)
                nc.vector.scalar_tensor_tensor(ct[:], ct[:], -9.0, ct[:],
                                                op0=Alu.add, op1=Alu.mult)
                nc.vector.tensor_scalar(ct[:], ct[:], 0.0, None,
                                        op0=Alu.is_equal, op1=Alu.add,
                                        accum_out=accU[:, i:i + 1])
                nc.vector.scalar_tensor_tensor(outv[:, i * W:(i + 1) * W],
                                               ct[:], 2.0, out01[:],
                                               op0=Alu.mult, op1=Alu.max,
                                               accum_out=accO[:, i:i + 1])

            # ---- per-core masked sums [P,4]: sumB sumW cntB cntW ----
            acc = pers.tile([P, 4], f32)
            scratch = vpool.tile([P, NPW], f32, tag="V")
            nc.vector.scalar_tensor_tensor(scratch[:], outv[:], 0.0, vals[:],
                                           op0=Alu.is_equal, op1=Alu.mult,
                                           accum_out=acc[:, 0:1])
            scratch2 = epool.tile([P, NPW], f32, tag="E")
            nc.vector.scalar_tensor_tensor(scratch2[:], outv[:], 1.0, vals[:],
                                           op0=Alu.is_equal, op1=Alu.mult,
                                           accum_out=acc[:, 1:2])
            cntU = pers.tile([P, 1], f32)
            nc.vector.tensor_reduce(cntU[:], accU[:], axis=mybir.AxisListType.X,
                                    op=Alu.add)
            sumO = pers.tile([P, 1], f32)
            nc.vector.tensor_reduce(sumO[:], accO[:], axis=mybir.AxisListType.X,
                                    op=Alu.add)
            # cntW = sumO - 2*cntU ; cntB = NPW - sumO + cntU
            nc.vector.tensor_scalar(acc[:, 3:4], cntU[:], -2.0, sumO[:],
                                    op0=Alu.mult, op1=Alu.add)
            nc.vector.tensor_scalar(acc[:, 2:3], cntU[:], sumO[:], float(NPW),
                                    op0=Alu.subtract, op1=Alu.add)

            # ---- AllReduce the [P,4] partials across cores ----
            cc_in = dram.tile([P, 4], f32)
            cc_out = dram.tile([P, 4], f32, addr_space="Shared")
            nc.sync.dma_start(cc_in[:], acc[:])
            if stub_cc:
                gsrc = cc_in
            else:
                nc.gpsimd.collective_compute(
                    "AllReduce", Alu.add,
                    replica_groups=[list(range(N_CORES))],
                    ins=[cc_in[:].opt()], outs=[cc_out[:].opt()])
                gsrc = cc_out

            # flatten [P,4] -> [1,512] and reduce over partitions on DVE
            gflat = pers.tile([1, P * 4], f32)
            nc.sync.dma_start(
                gflat[:],
                gsrc[:].rearrange("(o p) q -> o (p q)", o=1))
            g = pers.tile([1, 4], f32)
            nc.vector.tensor_reduce(
                g[:], gflat[:].rearrange("o (p q) -> o q p", q=4),
                axis=mybir.AxisListType.X, op=Alu.add)

            # avg = sum/max(cnt,1); m = (avgB+avgW)/2 ; s = avgW-avgB
            # scale/bias for phase 2: w1 = s*v - m*s
            cnt = pers.tile([1, 2], f32)
            nc.vector.tensor_scalar(cnt[:], g[:, 2:4], 1.0, None, op0=Alu.max)
            rc = pers.tile([1, 2], f32)
            nc.vector.reciprocal(rc[:], cnt[:])
            avg = pers.tile([1, 2], f32)
            nc.vector.tensor_tensor(avg[:], g[:, 0:2], rc[:], op=Alu.mult)
            mmid = pers.tile([1, 1], f32)
            nc.vector.tensor_scalar(mmid[:], avg[:, 0:1], avg[:, 1:2], 0.5,
                                    op0=Alu.add, op1=Alu.mult)
            sc = pers.tile([1, 2], f32)
            nc.vector.tensor_scalar(sc[:, 0:1], avg[:, 1:2], avg[:, 0:1], None,
                                    op0=Alu.subtract)
            nc.vector.tensor_scalar(sc[:, 1:2], mmid[:], sc[:, 0:1], -1.0,
                                    op0=Alu.mult, op1=Alu.mult)

            # broadcast (s, -m*s) to all partitions via PE ones-matmul
            pb = psum.tile([P, 2], f32)
            nc.tensor.matmul(pb[:], ones[:], sc[:], start=True, stop=True)
            scb = pers.tile([P, 2], f32)
            nc.vector.tensor_copy(scb[:], pb[:])

            if DEBUG_PROBES:
                nc.sync.dma_start(d_outv.ap(), outv[:])
                nc.sync.dma_start(d_acc.ap(), acc[:])
                nc.sync.dma_start(d_g.ap(), g[:])
                nc.sync.dma_start(d_scb.ap(), scb[:])

            # ---- phase 2: corr = outv + (outv==2)*((w1>=0) - 2) ----
            for k in range(PH2_CHUNKS):
                sl = slice(k * PH2_W, (k + 1) * PH2_W)
                w1 = spool.tile([P, PH2_W], f32, tag="w1")
                nc.scalar.activation(w1[:], vals[:, sl], Act.Identity,
                                     bias=scb[:, 1:2], scale=scb[:, 0:1])
                nc.vector.tensor_scalar(w1[:], w1[:], 0.0, -2.0,
                                        op0=Alu.is_ge, op1=Alu.add)
                nc.vector.scalar_tensor_tensor(w1[:], outv[:, sl], 2.0, w1[:],
                                               op0=Alu.is_equal, op1=Alu.mult)
                nc.vector.tensor_tensor(w1[:], outv[:, sl], w1[:],
                                        op=Alu.add)
                nc.sync.dma_start(out_f[:, sl], w1[:])

    nc.compile()
    return nc


def _get_nc():
    if "nc" not in _CACHE:
        _CACHE["nc"] = _build()
    return _CACHE["nc"]


def run(image, edges_prob, gt=None, trace=False, tmpdir=None):
    nc = _get_nc()
    img = np.ascontiguousarray(np.asarray(image), dtype=np.float32)
    edg = np.ascontiguousarray(np.asarray(edges_prob), dtype=np.float32)
    img = img.reshape(N_PATCH, NINE)
    edg = edg.reshape(N_PATCH, NINE)
    in_maps = []
    for c in range(N_CORES):
        sl = slice(c * NP_CORE, (c + 1) * NP_CORE)
        in_maps.append({"img": img[sl], "edg": edg[sl]})
    res = bass_utils.run_bass_kernel_spmd(
        nc, in_maps, core_ids=list(range(N_CORES)),
        trace=trace, tmpdir=tmpdir)
    shards = []
    for c in range(N_CORES):
        # undo the partition-contiguous layout: element (p*NPW + j) is
        # patch p*NPW + j of the shard -> flat order is already correct.
        shards.append(res.results[c]["out"])
    full = np.concatenate(shards).reshape(H_OUT, H_OUT)
    return full, res


def kernel(image, edges_prob, gt=None, **_ignored):
    full, _ = run(image, edges_prob, gt)
    return full


def _numpy_model(image, edges_prob):
    img = np.asarray(image).reshape(N_PATCH, NINE)
    edg = np.asarray(edges_prob).reshape(N_PATCH, NINE)
    m = edg > 0.5
    c = m.sum(1)
    sd = (img * m).sum(1)
    sall = img.sum(1)
    out01 = (9.0 * sd <= c * sall).astype(np.float32)
    outv = np.where((c == 0) | (c == 9), 2.0, out01)
    v = img[:, 4]
    mb = outv == 0.0
    mw = outv == 1.0
    avgB = (v * mb).sum() / max(mb.sum(), 1)
    avgW = (v * mw).sum() / max(mw.sum(), 1)
    cls = (np.abs(v - avgB) >= np.abs(v - avgW)).astype(np.float32)
    corr = np.where(outv == 2.0, cls, outv)
    return corr.reshape(H_OUT, H_OUT)


# revision 45
# speedup vs baseline: 1.0058x; 1.0058x over previous
"""Trainium2 Bass kernel for nn_Predictor_67585605370461 (segment_reduce).

Per patch (N=4194304, 9 elements each):
  m_dis = edg > 0.5 ; m_acc = (edg != 0) & ~m_dis
  md = mean(img | m_dis), ma = mean(img | m_acc)
  out = valid ? (md > ma ? 0 : 1) : 2
Global: avgB/avgW = masked means of center pixels over out==0 / out==1
  corr = out==2 ? (|v-avgB| < |v-avgW| ? 0 : 1) : out  -> reshape [2048,2048]

Key reductions used by the kernel:
  * md > ma  <=>  9*sd > c*sall   (sd = masked sum, c = count, sall = 9-sum)
  * exact zeros in the uniform edge data are measure-zero -> m_acc = ~m_dis
  * |v-avgB| >= |v-avgW|  <=>  (v - (avgB+avgW)/2) * (avgW-avgB) >= 0

Sharding: data-parallel over the patch axis, 524288 patches per core; the
four global scalars (sumB, sumW, cntB, cntW) go through one tiny AllReduce.
"""

import contextlib

import numpy as np

import concourse.bass as bass
import concourse.bacc as bacc
import concourse.mybir as mybir
import concourse.tile as tile
from concourse import bass_utils

N_CORES = 8
N_PATCH = 4194304
NP_CORE = N_PATCH // N_CORES  # 524288
P = 128
NINE = 9
W = 512                      # patches per partition-row per tile
F = W * NINE                 # 2304 free elements per tile
NTILES = NP_CORE // (P * W)  # 16
NPW = NTILES * W             # 4096 patches per partition per core
H_OUT = 2048
PH2_CHUNKS = 2
PH2_W = NPW // PH2_CHUNKS    # 1024

f32 = mybir.dt.float32
Alu = mybir.AluOpType
Act = mybir.ActivationFunctionType

_CACHE = {}
DEBUG_PROBES = False


def _build(stub_cc=False):
    num_devices = 1 if stub_cc else N_CORES
    nc = bacc.Bacc("TRN2", target_bir_lowering=False, debug=False,
                   num_devices=num_devices)
    img = nc.dram_tensor("img", [NP_CORE, NINE], f32, kind="ExternalInput")
    edg = nc.dram_tensor("edg", [NP_CORE, NINE], f32, kind="ExternalInput")
    out = nc.dram_tensor("out", [NP_CORE], f32, kind="ExternalOutput")
    if DEBUG_PROBES:
        d_outv = nc.dram_tensor("d_outv", [P, NPW], f32, kind="ExternalOutput")
        d_acc = nc.dram_tensor("d_acc", [P, 4], f32, kind="ExternalOutput")
        d_g = nc.dram_tensor("d_g", [1, 4], f32, kind="ExternalOutput")
        d_scb = nc.dram_tensor("d_scb", [P, 2], f32, kind="ExternalOutput")

    # partition-contiguous patch layout: partition p owns patches
    # [p*NPW, (p+1)*NPW) of this core's shard; tile i covers window
    # [i*W, (i+1)*W) of every partition.  Output is then one [P, NPW] DMA.
    img_t = img.ap().rearrange("(p i w) n -> i p (w n)", p=P, i=NTILES, w=W)
    edg_t = edg.ap().rearrange("(p i w) n -> i p (w n)", p=P, i=NTILES, w=W)
    out_f = out.ap().rearrange("(p j) -> p j", p=P)

    with tile.TileContext(nc) as tc:
        with (
            tc.tile_pool(name="vin", bufs=3) as vpool,
            tc.tile_pool(name="ein", bufs=3) as epool,
            tc.tile_pool(name="l1", bufs=2) as l1pool,
            tc.tile_pool(name="small", bufs=2) as spool,
            tc.tile_pool(name="red", bufs=3) as redpool,
            tc.tile_pool(name="persist", bufs=1) as pers,
            tc.tile_pool(name="psum", bufs=1, space="PSUM") as psum,
            tc.tile_pool(name="dram", bufs=1, space="DRAM") as dram,
        ):
            outv = pers.tile([P, NPW], f32)
            vals = pers.tile([P, NPW], f32)
            accU = pers.tile([P, NTILES], f32)   # per-tile unknown counts
            accO = pers.tile([P, NTILES], f32)   # per-tile sum of outv
            nbias = pers.tile([P, 1], f32)
            nc.vector.memset(nbias[:], -0.5)
            ones = pers.tile([1, P], f32)
            nc.vector.memset(ones[:], 1.0)

            for i in range(NTILES):
                Vt = vpool.tile([P, F], f32, tag="V")
                nc.sync.dma_start(Vt[:], img_t[i])
                Et = epool.tile([P, F], f32, tag="E")
                nc.sync.dma_start(Et[:], edg_t[i])

                # ACT: mask = sign(relu(E - 0.5)), computed in place on Et
                nc.scalar.activation(Et[:], Et[:], Act.Relu, bias=nbias[:])
                nc.scalar.sign(Et[:], Et[:])

                v3 = Vt[:].rearrange("p (w n) -> p w n", n=NINE)
                m3 = Et[:].rearrange("p (w n) -> p w n", n=NINE)

                # reads of the original V first: 9-sum + center pixel
                sall = redpool.tile([P, W], f32, tag="sall")
                nc.vector.tensor_reduce(
                    sall[:], v3, axis=mybir.AxisListType.X, op=Alu.add)
                nc.scalar.copy(vals[:, i * W:(i + 1) * W], v3[:, :, 4])

                # count tree: c = sum of 9 mask bits (L1 on gpsimd)
                l1 = l1pool.tile([P, W, 4], f32, tag="l1")
                nc.gpsimd.tensor_tensor(l1[:], m3[:, :, 0:4], m3[:, :, 4:8],
                                        op=Alu.add)

                # GpSimd: masked values, in place on Vt
                nc.gpsimd.tensor_tensor(Vt[:], Et[:], Vt[:], op=Alu.mult)
                sd = redpool.tile([P, W], f32, tag="sd")
                nc.vector.tensor_reduce(
                    sd[:], v3, axis=mybir.AxisListType.X, op=Alu.add)

                # tree folded into l1 in place; t1 = c*sall onto sall
                nc.vector.tensor_tensor(l1[:, :, 0:2], l1[:, :, 0:2],
                                        l1[:, :, 2:4], op=Alu.add)
                nc.vector.tensor_tensor(l1[:, :, 0:1], l1[:, :, 0:1],
                                        l1[:, :, 1:2], op=Alu.add)
                ct = redpool.tile([P, W], f32, tag="ct")
                nc.vector.tensor_tensor(ct[:], l1[:, :, 0], m3[:, :, 8],
                                        op=Alu.add)

                # classification: out01 = (9*sd <= c*sall)
                # invalid iff (c-9)*c == 0
                nc.vector.tensor_tensor(sall[:], ct[:], sall[:], op=Alu.mult)
                out01 = redpool.tile([P, W], f32, tag="out01")
                nc.vector.scalar_tensor_tensor(out01[:], sd[:], 9.0, sall[:],
                                               op0=Alu.mult, op1=Alu.is_le)
                nc.vector.scalar_tensor_tensor(ct[:], ct[:], -9.0, ct[:],
                                                op0=Alu.add, op1=Alu.mult)
                nc.vector.tensor_scalar(ct[:], ct[:], 0.0, None,
                                        op0=Alu.is_equal, op1=Alu.add,
                                        accum_out=accU[:, i:i + 1])
                nc.vector.scalar_tensor_tensor(outv[:, i * W:(i + 1) * W],
                                               ct[:], 2.0, out01[:],
                                               op0=Alu.mult, op1=Alu.max,
                                               accum_out=accO[:, i:i + 1])

            # ---- per-core masked sums [P,4]: sumB sumW cntB cntW ----
            acc = pers.tile([P, 4], f32)
            scratch = vpool.tile([P, NPW], f32, tag="V")
            nc.vector.scalar_tensor_tensor(scratch[:], outv[:], 0.0, vals[:],
                                           op0=Alu.is_equal, op1=Alu.mult,
                                           accum_out=acc[:, 0:1])
            scratch2 = epool.tile([P, NPW], f32, tag="E")
            nc.vector.scalar_tensor_tensor(scratch2[:], outv[:], 1.0, vals[:],
                                           op0=Alu.is_equal, op1=Alu.mult,
                                           accum_out=acc[:, 1:2])
            cntU = pers.tile([P, 1], f32)
            nc.vector.tensor_reduce(cntU[:], accU[:], axis=mybir.AxisListType.X,
                                    op=Alu.add)
            sumO = pers.tile([P, 1], f32)
            nc.vector.tensor_reduce(sumO[:], accO[:], axis=mybir.AxisListType.X,
                                    op=Alu.add)
            # cntW = sumO - 2*cntU ; cntB = NPW - sumO + cntU
            nc.vector.tensor_scalar(acc[:, 3:4], cntU[:], -2.0, sumO[:],
                                    op0=Alu.mult, op1=Alu.add)
            nc.vector.tensor_scalar(acc[:, 2:3], cntU[:], sumO[:], float(NPW),
                                    op0=Alu.subtract, op1=Alu.add)

            # ---- AllReduce the [P,4] partials across cores ----
            cc_in = dram.tile([P, 4], f32)
            cc_out = dram.tile([P, 4], f32, addr_space="Shared")
            nc.sync.dma_start(cc_in[:], acc[:])
            if stub_cc:
                gsrc = cc_in
            else:
                nc.gpsimd.collective_compute(
                    "AllReduce", Alu.add,
                    replica_groups=[list(range(N_CORES))],
                    ins=[cc_in[:].opt()], outs=[cc_out[:].opt()])
                gsrc = cc_out

            # flatten [P,4] -> [1,512] and reduce over partitions on DVE
            gflat = pers.tile([1, P * 4], f32)
            nc.sync.dma_start(
                gflat[:],
                gsrc[:].rearrange("(o p) q -> o (p q)", o=1))
            g = pers.tile([1, 4], f32)
            nc.vector.tensor_reduce(
                g[:], gflat[:].rearrange("o (p q) -> o q p", q=4),
                axis=mybir.AxisListType.X, op=Alu.add)

            # avg = sum/max(cnt,1); m = (avgB+avgW)/2 ; s = avgW-avgB
            # scale/bias for phase 2: w1 = s*v - m*s
            cnt = pers.tile([1, 2], f32)
            nc.vector.tensor_scalar(cnt[:], g[:, 2:4], 1.0, None, op0=Alu.max)
            rc = pers.tile([1, 2], f32)
            nc.vector.reciprocal(rc[:], cnt[:])
            avg = pers.tile([1, 2], f32)
            nc.vector.tensor_tensor(avg[:], g[:, 0:2], rc[:], op=Alu.mult)
            mmid = pers.tile([1, 1], f32)
            nc.vector.tensor_scalar(mmid[:], avg[:, 0:1], avg[:, 1:2], 0.5,
                                    op0=Alu.add, op1=Alu.mult)
            sc = pers.tile([1, 2], f32)
            nc.vector.tensor_scalar(sc[:, 0:1], avg[:, 1:2], avg[:, 0:1], None,
                                    op0=Alu.subtract)
            nc.vector.tensor_scalar(sc[:, 1:2], mmid[:], sc[:, 0:1], -1.0,
                                    op0=Alu.mult, op1=Alu.mult)

            # broadcast (s, -m*s) to all partitions via PE ones-matmul
            pb = psum.tile([P, 2], f32)
            nc.tensor.matmul(pb[:], ones[:], sc[:], start=True, stop=True)
            scb = pers.tile([P, 2], f32)
            nc.vector.tensor_copy(scb[:], pb[:])

            if DEBUG_PROBES:
                nc.sync.dma_start(d_outv.ap(), outv[:])
                nc.sync.dma_start(d_acc.ap(), acc[:])
                nc.sync.dma_start(d_g.ap(), g[:])
                nc.sync.dma_start(d_scb.ap(), scb[:])

            # ---- phase 2: corr = outv + (outv==2)*((w1>=0) - 2) ----
            for k in range(PH2_CHUNKS):
                sl = slice(k * PH2_W, (k + 1) * PH2_W)
                w1 = spool.tile([P, PH2_W], f32, tag="w1")
                nc.scalar.activation(w1[:], vals[:, sl], Act.Identity,
                                     bias=scb[:, 1:2], scale=scb[:, 0:1])
                nc.vector.tensor_scalar(w1[:], w1[:], 0.0, -2.0,
                                        op0=Alu.is_ge, op1=Alu.add)
                nc.vector.scalar_tensor_tensor(w1[:], outv[:, sl], 2.0, w1[:],
                                               op0=Alu.is_equal, op1=Alu.mult)
                nc.vector.tensor_tensor(w1[:], outv[:, sl], w1[:],
                                        op=Alu.add)
                nc.sync.dma_start(out_f[:, sl], w1[:])

    nc.compile()
    return nc


def _get_nc():
    if "nc" not in _CACHE:
        _CACHE["nc"] = _build()
    return _CACHE["nc"]


def run(image, edges_prob, gt=None, trace=False, tmpdir=None):
    nc = _get_nc()
    img = np.ascontiguousarray(np.asarray(image), dtype=np.float32)
    edg = np.ascontiguousarray(np.asarray(edges_prob), dtype=np.float32)
    img = img.reshape(N_PATCH, NINE)
    edg = edg.reshape(N_PATCH, NINE)
    in_maps = []
    for c in range(N_CORES):
        sl = slice(c * NP_CORE, (c + 1) * NP_CORE)
        in_maps.append({"img": img[sl], "edg": edg[sl]})
    res = bass_utils.run_bass_kernel_spmd(
        nc, in_maps, core_ids=list(range(N_CORES)),
        trace=trace, tmpdir=tmpdir)
    shards = []
    for c in range(N_CORES):
        # undo the partition-contiguous layout: element (p*NPW + j) is
        # patch p*NPW + j of the shard -> flat order is already correct.
        shards.append(res.results[c]["out"])
    full = np.concatenate(shards).reshape(H_OUT, H_OUT)
    return full, res


def kernel(image, edges_prob, gt=None, **_ignored):
    full, _ = run(image, edges_prob, gt)
    return full


def _numpy_model(image, edges_prob):
    img = np.asarray(image).reshape(N_PATCH, NINE)
    edg = np.asarray(edges_prob).reshape(N_PATCH, NINE)
    m = edg > 0.5
    c = m.sum(1)
    sd = (img * m).sum(1)
    sall = img.sum(1)
    out01 = (9.0 * sd <= c * sall).astype(np.float32)
    outv = np.where((c == 0) | (c == 9), 2.0, out01)
    v = img[:, 4]
    mb = outv == 0.0
    mw = outv == 1.0
    avgB = (v * mb).sum() / max(mb.sum(), 1)
    avgW = (v * mw).sum() / max(mw.sum(), 1)
    cls = (np.abs(v - avgB) >= np.abs(v - avgW)).astype(np.float32)
    corr = np.where(outv == 2.0, cls, outv)
    return corr.reshape(H_OUT, H_OUT)
